# revision 26
# baseline (speedup 1.0000x reference)
"""AffinityPropagate Trainium2 kernel.

Reference computation (per batch element):
    k_d = softmax(guided_d, axis=channel)          d = 1,2,3 (dilations)
    repeat 8 times:
        o_d = sum_ch k_d[ch] * shift(x, offset(d, ch))
        x   = o_1*fuse[0] + o_2*fuse[1] + o_3*fuse[2]

Strategy: pure data parallel over the batch (8 batches -> 8 NeuronCores).
Per core, the three 9-tap dilated kernels are pre-fused with the fuse
weights into 25 distinct-offset weight fields (the three (0,0) taps
share one field) stored fp16 in ONE [120, 25, 4, 640] SBUF tile, slot
order grouped by row-offset dh so each iteration needs only 8 DVE
tensor_tensor ops (one per dh-group, multi-tap strided APs) instead of
25 -- the per-op 151-cycle overhead and semaphore traffic shrink 3x.
x is kept in a halo layout: partition p owns image rows [4p, 4p+4),
stored with 3 halo rows each side and 4 zero border columns each side
([120, 10, 648] fp16).

Each iteration: per dh-group, VectorE multiplies the weight slots with
a strided window group of x (fp16, 2x DVE mode); TensorE accumulates
the products into PSUM fp32 via identity-stationary matmuls; ScalarE
evacuates PSUM back to the fp16 x buffer.  Halo rows are rebuilt by
TensorE with shift-by-one-partition matmuls.  dh=0 groups are emitted
first so they overlap the halo rebuild.

Setup streams the guided tensors on two DMA queue sets at once: odd
channels ride the gpsimd SWDGE queue with inline f32->f16 cast
(halves SBUF staging + ScalarE exp cost), even channels the sync
HWDGE queue as f32; x and the x-halo-init SBUF->SBUF DMAs ride the
otherwise idle scalar (Act) HWDGE ring; fuse is cast-DMAd on gpsimd.
Iteration-1 taps of each dilation are emitted between the setup
stages so they execute under the ~110us DMA stream (38 MB at the
~358 GB/s HBM-per-core limit).

GpSimd tensor ops stay off the tap path: DVE's tensor_tensor holds
the shared DVE/GpSimd SBUF port, so concurrent GpSimd tensor work
hard-blocks DVE (measured 1.5-3x slowdown in a prior session).
"""

import numpy as np

import concourse.bacc as bacc
import concourse.bass as bass
import concourse.mybir as mybir
from concourse.bass_utils import run_bass_kernel_spmd
from concourse.masks import make_identity
from concourse.tile import TileContext

H, W = 480, 640
P = 120          # partitions used (each owns R rows)
R = 4            # rows per partition
HALO = 3         # halo rows each side
CB = 4           # border cols each side
ROWB = R + 2 * HALO          # 10 buffer rows per partition
COLB = W + 2 * CB            # 648 buffer cols
NFLAT = ROWB * COLB
RW = R * W                   # 2560 elems per field per partition
PROP_TIME = 8
NCORES = 8

F16 = mybir.dt.float16
F32 = mybir.dt.float32

# Weight slot layout: groups by row offset dh; within a group the col
# offsets dw form an arithmetic progression so one strided AP covers
# the whole group.  All groups <=3 taps so the m scratch tiles stay
# 15KB and bufs=3 gives DVE three groups of runway over PE.
# (dh, dw0, step, ntap, slot0)
GROUPS = [
    (-3, -3, 3, 3, 0),
    (-2, -2, 2, 3, 3),
    (-1, -1, 1, 3, 6),
    (0, -3, 1, 2, 9),
    (0, -1, 1, 1, 11),
    (0, 0, 1, 1, 12),    # merged (0,0) center, emitted last
    (0, 1, 1, 3, 13),
    (1, -1, 1, 3, 16),
    (2, -2, 2, 3, 19),
    (3, -3, 3, 3, 22),
]
# emission order per iteration: halo-independent dh=0 groups first
# (cover the in-flight halo DMAs), the 2-tap group second-to-last so
# few matmuls drain after the final products, and the single-tap
# center group LAST: only 5 matmuls separate its product from the
# PSUM evacuation.
GROUP_ORDER = [4, 6, 2, 7, 1, 8, 0, 9, 3]
LAST_GROUP = 5

CENTER_SLOT = 12


def _tap_table():
    field_of = {}
    for dh, dw0, step, ntap, slot0 in GROUPS:
        for t in range(ntap):
            field_of[(dh, dw0 + t * step)] = slot0 + t
    assert len(field_of) == 25
    assert field_of[(0, 0)] == CENTER_SLOT
    return field_of


FIELD_OF = _tap_table()


def build_nc():
    nc = bacc.Bacc("TRN2", target_bir_lowering=False, debug=False)

    g_dram = [
        nc.dram_tensor(name, [9, H, W], F32, kind="ExternalInput")
        for name in ("guided1", "guided2", "guided3")
    ]
    fuse_dram = nc.dram_tensor("fuse", [3, H, W], F32, kind="ExternalInput")
    x_dram = nc.dram_tensor("x", [1, H, W], F32, kind="ExternalInput")
    out_dram = nc.dram_tensor("out", [1, H, W], F32, kind="ExternalOutput")

    # DRAM access pattern: partition p <- rows [4p, 4p+4) (one
    # contiguous 10KB descriptor per partition)
    def rows_ap(t, extra_off=0):
        return bass.AP(t, extra_off, [[RW, P], [1, RW]])

    uid = [0]

    def nm(pfx):
        uid[0] += 1
        return f"{pfx}{uid[0]}"

    with TileContext(nc) as tc:
        with (
            tc.tile_pool(name="const", bufs=1) as constp,
            tc.tile_pool(name="wpool", bufs=1) as wpool,
            tc.tile_pool(name="xa", bufs=1) as xap,
            tc.tile_pool(name="psit", bufs=1, space="PSUM") as psi,
        ):
            ident = constp.tile([P, P], F16)
            make_identity(nc, ident)

            W_ALL = wpool.tile([P, 25, R, W], F16, tag="wall", name="wall")
            XA = xap.tile([P, ROWB, COLB], F16, tag="XA")

            def slot3(s):
                return W_ALL[:, s]

            def slotf(s):
                return W_ALL[:, s].rearrange("p a b -> p (a b)")

            def group_win(Xc, dh, dw0, step, ntap):
                """[P, ntap, R, W] AP over the x buffer: tap t reads the
                window shifted by (dh, dw0 + t*step)."""
                w = Xc[:, HALO + dh:HALO + dh + R, CB + dw0:CB + dw0 + W]
                return bass.AP(
                    w.tensor, w.offset,
                    [list(w.ap[0]), [step, ntap], [COLB, R], [1, W]],
                )

            mm_n = [0]

            def acc(ps, mflat, nchunk):
                """Accumulate nchunk 512-col chunks of mflat into ps.
                The first 5 chunks of an iteration (one per PSUM bank
                column) reset the accumulation (start=True); the stop
                is always emitted by emit_tail_fused."""
                for k in range(nchunk):
                    first = mm_n[0] < 5
                    pcol = (k * 512) % RW
                    nc.tensor.matmul(
                        out=ps[:, pcol:pcol + 512],
                        lhsT=ident,
                        rhs=mflat[:, k * 512:(k + 1) * 512],
                        start=first,
                        stop=False,
                    )
                    mm_n[0] += 1

            HB3 = HALO * COLB      # 1944, halo span per side
            HB = RW // 2           # 1280, half-field

            def emit_tail_fused(mflat, ntap, ps, Xn, last_iter):
                """Emit the last (single-tap center) group's 5 matmuls,
                evacuate PSUM in halves on Vector + Scalar concurrently
                (DVE is idle at the boundary), then rebuild the halo
                rows with partition-shifted SBUF->SBUF DMAs on the
                otherwise idle sync/gpsimd rings -- PE and Scalar stay
                off the halo path entirely.  PSUM dep tracking is
                tile-granular, so the evacuation strictly follows all
                matmuls -- keep the matmul count after the last tap
                product minimal (5)."""
                for b in range(5):
                    for t in range(ntap):
                        nc.tensor.matmul(
                            out=ps[:, b * 512:(b + 1) * 512],
                            lhsT=ident,
                            rhs=mflat[:, t * RW + b * 512:
                                      t * RW + (b + 1) * 512],
                            start=False,
                            stop=(b == 4 and t == ntap - 1),
                        )
                        mm_n[0] += 1
                if last_iter:
                    return
                ps3 = ps.rearrange("p (a b) -> p a b", a=R)
                HR = R // 2
                nc.vector.tensor_copy(
                    out=Xn[:, HALO + HR:HALO + R, CB:CB + W],
                    in_=ps3[:, HR:R],
                )
                nc.scalar.copy(
                    out=Xn[:, HALO:HALO + HR, CB:CB + W],
                    in_=ps3[:, 0:HR],
                )
                Xn_f = Xn.rearrange("p a b -> p (a b)")
                nc.sync.dma_start(
                    out=Xn_f[1:P, 0:HB3],
                    in_=Xn_f[0:P - 1, (HALO + 1) * COLB:
                             (HALO + 1) * COLB + HB3],
                )
                nc.gpsimd.dma_start(
                    out=Xn_f[0:P - 1, (R + HALO) * COLB:NFLAT],
                    in_=Xn_f[1:P, HALO * COLB:HALO * COLB + HB3],
                )

            # ---------------- setup + iteration 1 ----------------
            ps0 = psi.tile([P, RW], F32, tag="ps", name="ps_it0")
            with (
                tc.tile_pool(name="syncg", bufs=1) as sgp,
                tc.tile_pool(name="gpg", bufs=4) as ggp,
                tc.tile_pool(name="fusep", bufs=1) as fusep,
                tc.tile_pool(name="m0pool", bufs=3) as m0pool,
                tc.tile_pool(name="psst", bufs=1, space="PSUM") as psp,
            ):
                nc.vector.memset(XA, 0.0)
                for d_idx in range(3):
                    d = d_idx + 1
                    dil_slots = [FIELD_OF[((ch // 3 - 1) * d, (ch % 3 - 1) * d)]
                                 for ch in range(9) if ch != 4]

                    fuse16 = fusep.tile([P, RW], F16, tag="f16",
                                        name=nm("f16_"))
                    nc.gpsimd.dma_start(
                        out=fuse16, in_=rows_ap(fuse_dram, d_idx * H * W)
                    )
                    e_c = fusep.tile([P, R, W], F16, tag="ec", name=nm("ec_"))
                    for ch in range(9):
                        if ch == 4:
                            dest = e_c.rearrange("p a b -> p (a b)")
                        else:
                            s = FIELD_OF[((ch // 3 - 1) * d, (ch % 3 - 1) * d)]
                            dest = slotf(s)
                        g = ggp.tile([P, RW], F16, tag="g16",
                                     name=nm("g16_"))
                        nc.gpsimd.dma_start(
                            out=g, in_=rows_ap(g_dram[d_idx], ch * H * W)
                        )
                        nc.scalar.activation(
                            out=dest, in_=g,
                            func=mybir.ActivationFunctionType.Exp,
                        )
                    if d_idx == 0:
                        # x load + halo-init ride the idle Act HWDGE ring
                        xs32 = sgp.tile([P, RW], F32, tag="g")
                        nc.scalar.dma_start(out=xs32, in_=rows_ap(x_dram))
                        nc.vector.tensor_copy(
                            out=XA[:, HALO:HALO + R, CB:CB + W],
                            in_=xs32.rearrange("p (a b) -> p a b", a=R),
                        )
                        XA_f = XA.rearrange("p a b -> p (a b)")
                        nc.scalar.dma_start(
                            out=XA_f[1:P, 0:HALO * COLB],
                            in_=XA_f[0:P - 1, R * COLB:(R + HALO) * COLB],
                        )
                        nc.scalar.dma_start(
                            out=XA_f[0:P - 1, (R + HALO) * COLB:NFLAT],
                            in_=XA_f[1:P, HALO * COLB:2 * HALO * COLB],
                        )
                    # channel sums + 1/sum in halves (3 PSUM banks)
                    t_ = fusep.tile([P, RW], F16, tag="t", name=nm("t_"))
                    for h0 in (0, HB):
                        pss = psp.tile([P, HB], F32, tag="pss",
                                       name=nm("pss_"))
                        for ci, ch in enumerate(range(9)):
                            if ch == 4:
                                sf = e_c.rearrange("p a b -> p (a b)")
                            else:
                                sf = slotf(
                                    FIELD_OF[((ch // 3 - 1) * d,
                                              (ch % 3 - 1) * d)])
                            for c0 in range(0, HB, 512):
                                c1 = min(c0 + 512, HB)
                                nc.tensor.matmul(
                                    out=pss[:, c0:c1], lhsT=ident,
                                    rhs=sf[:, h0 + c0:h0 + c1],
                                    start=(ci == 0), stop=(ci == 8),
                                )
                        r_ = fusep.tile([P, HB], F32, tag="r", name=nm("r_"))
                        nc.vector.reciprocal_approx_fast(out=r_, in_=pss)
                        nc.vector.tensor_mul(
                            out=t_[:, h0:h0 + HB], in0=fuse16[:, h0:h0 + HB],
                            in1=r_,
                        )
                    tv = t_.rearrange("p (a b) -> p a b", a=R)
                    for s in dil_slots:
                        nc.vector.tensor_mul(out=slot3(s), in0=slot3(s),
                                             in1=tv)
                    if d_idx == 0:
                        nc.vector.tensor_mul(out=slot3(CENTER_SLOT),
                                             in0=e_c, in1=tv)
                    else:
                        nc.vector.tensor_mul(out=e_c, in0=e_c, in1=tv)
                        nc.vector.tensor_add(out=slot3(CENTER_SLOT),
                                             in0=slot3(CENTER_SLOT), in1=e_c)
                    # iteration-1 taps of this dilation, single-tap ops:
                    # DMA-hidden, small m0 footprint.  The center tap is
                    # deferred to the iteration section's fused tail.
                    taps = [((ch // 3 - 1) * d, (ch % 3 - 1) * d)
                            for ch in range(9) if ch != 4]
                    for dh, dw in taps:
                        m0 = m0pool.tile([P, R, W], F16, tag="m",
                                         name=nm(f"m0d{d}_"))
                        win = XA[:, HALO + dh:HALO + dh + R,
                                 CB + dw:CB + dw + W]
                        nc.vector.tensor_mul(
                            out=m0, in0=slot3(FIELD_OF[(dh, dw)]), in1=win,
                        )
                        acc(ps0, m0.rearrange("p a b -> p (a b)"), 5)
                assert mm_n[0] == 120

            # ---------------- iterations 2..8 ----------------
            with tc.tile_pool(name="xb", bufs=1) as xbp:
                XB = xbp.tile([P, ROWB, COLB], F16, tag="XB")
                # only the border cols need zeroing (the interior is
                # overwritten by evacuation, halo rows by the halo DMAs)
                # -- plus the image-edge halo rows the DMAs never touch
                nc.vector.memset(XB[:, :, 0:CB], 0.0)
                nc.vector.memset(XB[:, :, CB + W:COLB], 0.0)
                nc.vector.memset(XB[:, 0:HALO, :], 0.0)
                nc.vector.memset(XB[:, R + HALO:ROWB, :], 0.0)

                with tc.tile_pool(name="mpool", bufs=3) as mpool:
                    # iteration-1's deferred center tap + fused tail
                    mc = mpool.tile([P, 3, R, W], F16, tag="m",
                                    name=nm("mc_"))
                    nc.vector.tensor_mul(
                        out=mc[:, 0],
                        in0=slot3(CENTER_SLOT),
                        in1=XA[:, HALO:HALO + R, CB:CB + W],
                    )
                    emit_tail_fused(
                        mc[:, 0:1].rearrange("p t a b -> p (t a b)"),
                        1, ps0, XB, False)
                    assert mm_n[0] == 125

                    bufs = [XA, XB]
                    ps = ps0
                    for it in range(1, PROP_TIME):
                        Xc = bufs[it % 2]
                        Xn = bufs[(it + 1) % 2]
                        mm_n[0] = 0
                        ps = psi.tile([P, RW], F32, tag="ps",
                                      name=nm("ps_"))
                        for g in GROUP_ORDER:
                            dh, dw0, step, ntap, slot0 = GROUPS[g]
                            m = mpool.tile([P, 3, R, W], F16, tag="m",
                                           name=nm(f"m{it}_"))
                            msub = m[:, 0:ntap]
                            nc.vector.tensor_mul(
                                out=msub,
                                in0=W_ALL[:, slot0:slot0 + ntap],
                                in1=group_win(Xc, dh, dw0, step, ntap),
                            )
                            acc(ps,
                                msub.rearrange("p t a b -> p (t a b)"),
                                5 * ntap)
                        # last group: single-tap center -- only 5 matmuls
                        # separate its product from the evacuation
                        ml = mpool.tile([P, 3, R, W], F16, tag="m",
                                        name=nm(f"ml{it}_"))
                        nc.vector.tensor_mul(
                            out=ml[:, 0],
                            in0=slot3(CENTER_SLOT),
                            in1=Xc[:, HALO:HALO + R, CB:CB + W],
                        )
                        emit_tail_fused(
                            ml[:, 0:1].rearrange("p t a b -> p (t a b)"),
                            1, ps, Xn, it == PROP_TIME - 1)
                        assert mm_n[0] == 125

                with tc.tile_pool(name="stagep", bufs=1) as stagep:
                    stage = stagep.tile([P, RW], F32)
                    nc.scalar.copy(out=stage, in_=ps)
                    nc.sync.dma_start(out=rows_ap(out_dram), in_=stage)

    nc.compile()
    return nc


_NC = None


def _get_nc():
    global _NC
    if _NC is None:
        _NC = build_nc()
    return _NC


def _in_maps(guided1, guided2, guided3, fuse, x):
    maps = []
    for b in range(NCORES):
        maps.append({
            "guided1": np.ascontiguousarray(guided1[b], dtype=np.float32),
            "guided2": np.ascontiguousarray(guided2[b], dtype=np.float32),
            "guided3": np.ascontiguousarray(guided3[b], dtype=np.float32),
            "fuse": np.ascontiguousarray(fuse[b], dtype=np.float32),
            "x": np.ascontiguousarray(x[b], dtype=np.float32),
        })
    return maps


def kernel(guided1, guided2, guided3, fuse, x):
    nc = _get_nc()
    res = run_bass_kernel_spmd(
        nc, _in_maps(guided1, guided2, guided3, fuse, x),
        core_ids=list(range(NCORES)),
    )
    return np.stack([res.results[b]["out"] for b in range(NCORES)], axis=0)


def kernel_profiled(guided1, guided2, guided3, fuse, x):
    """Returns (output, BassKernelResults) with trace enabled."""
    nc = _get_nc()
    res = run_bass_kernel_spmd(
        nc, _in_maps(guided1, guided2, guided3, fuse, x),
        core_ids=list(range(NCORES)), trace=True,
    )
    out = np.stack([res.results[b]["out"] for b in range(NCORES)], axis=0)
    return out, res


# revision 30
# speedup vs baseline: 1.0451x; 1.0451x over previous
"""AffinityPropagate Trainium2 kernel.

Reference computation (per batch element):
    k_d = softmax(guided_d, axis=channel)          d = 1,2,3 (dilations)
    repeat 8 times:
        o_d = sum_ch k_d[ch] * shift(x, offset(d, ch))
        x   = o_1*fuse[0] + o_2*fuse[1] + o_3*fuse[2]

Strategy: pure data parallel over the batch (8 batches -> 8 NeuronCores).
Per core, the three 9-tap dilated kernels are pre-fused with the fuse
weights into 25 distinct-offset weight fields (the three (0,0) taps
share one field) stored fp16 in ONE [120, 25, 4, 640] SBUF tile, slot
order grouped by row-offset dh so each iteration needs only 8 DVE
tensor_tensor ops (one per dh-group, multi-tap strided APs) instead of
25 -- the per-op 151-cycle overhead and semaphore traffic shrink 3x.
x is kept in a halo layout: partition p owns image rows [4p, 4p+4),
stored with 3 halo rows each side and 4 zero border columns each side
([120, 10, 648] fp16).

Each iteration: per dh-group, VectorE multiplies the weight slots with
a strided window group of x (fp16, 2x DVE mode); TensorE accumulates
the products into PSUM fp32 via identity-stationary matmuls; ScalarE
evacuates PSUM back to the fp16 x buffer.  Halo rows are rebuilt by
TensorE with shift-by-one-partition matmuls.  dh=0 groups are emitted
first so they overlap the halo rebuild.

Setup streams the guided tensors on two DMA queue sets at once: odd
channels ride the gpsimd SWDGE queue with inline f32->f16 cast
(halves SBUF staging + ScalarE exp cost), even channels the sync
HWDGE queue as f32; x and the x-halo-init SBUF->SBUF DMAs ride the
otherwise idle scalar (Act) HWDGE ring; fuse is cast-DMAd on gpsimd.
Iteration-1 taps of each dilation are emitted between the setup
stages so they execute under the ~110us DMA stream (38 MB at the
~358 GB/s HBM-per-core limit).

GpSimd tensor ops stay off the tap path: DVE's tensor_tensor holds
the shared DVE/GpSimd SBUF port, so concurrent GpSimd tensor work
hard-blocks DVE (measured 1.5-3x slowdown in a prior session).
"""

import numpy as np

import concourse.bacc as bacc
import concourse.bass as bass
import concourse.mybir as mybir
from concourse.bass_utils import run_bass_kernel_spmd
from concourse.masks import make_identity
from concourse.tile import TileContext

H, W = 480, 640
P = 120          # partitions used (each owns R rows)
R = 4            # rows per partition
HALO = 3         # halo rows each side
CB = 4           # border cols each side
ROWB = R + 2 * HALO          # 10 buffer rows per partition
COLB = W + 2 * CB            # 648 buffer cols
NFLAT = ROWB * COLB
RW = R * W                   # 2560 elems per field per partition
PROP_TIME = 8
NCORES = 8

F16 = mybir.dt.float16
F32 = mybir.dt.float32

# Weight slot layout: groups by row offset dh; within a group the col
# offsets dw form an arithmetic progression so one strided AP covers
# the whole group.  All groups <=3 taps so the m scratch tiles stay
# 15KB and bufs=3 gives DVE three groups of runway over PE.
# (dh, dw0, step, ntap, slot0)
GROUPS = [
    (-3, -3, 3, 3, 0),
    (-2, -2, 2, 3, 3),
    (-1, -1, 1, 3, 6),
    (0, -3, 1, 2, 9),
    (0, -1, 1, 1, 11),
    (0, 0, 1, 1, 12),    # merged (0,0) center, emitted last
    (0, 1, 1, 3, 13),
    (1, -1, 1, 3, 16),
    (2, -2, 2, 3, 19),
    (3, -3, 3, 3, 22),
]
# emission order per iteration: halo-independent dh=0 groups first
# (cover the halo rebuild), the single-tap center group LAST so only
# 5 matmuls separate the final product from the PSUM evacuation.
GROUP_ORDER = [4, 6, 3, 2, 7, 1, 8, 0, 9]
LAST_GROUP = 5

CENTER_SLOT = 12


def _tap_table():
    field_of = {}
    for dh, dw0, step, ntap, slot0 in GROUPS:
        for t in range(ntap):
            field_of[(dh, dw0 + t * step)] = slot0 + t
    assert len(field_of) == 25
    assert field_of[(0, 0)] == CENTER_SLOT
    return field_of


FIELD_OF = _tap_table()


def build_nc():
    nc = bacc.Bacc("TRN2", target_bir_lowering=False, debug=False)

    g_dram = [
        nc.dram_tensor(name, [9, H, W], F32, kind="ExternalInput")
        for name in ("guided1", "guided2", "guided3")
    ]
    fuse_dram = nc.dram_tensor("fuse", [3, H, W], F32, kind="ExternalInput")
    x_dram = nc.dram_tensor("x", [1, H, W], F32, kind="ExternalInput")
    out_dram = nc.dram_tensor("out", [1, H, W], F32, kind="ExternalOutput")

    # DRAM access pattern: partition p <- rows [4p, 4p+4) (one
    # contiguous 10KB descriptor per partition)
    def rows_ap(t, extra_off=0):
        return bass.AP(t, extra_off, [[RW, P], [1, RW]])

    uid = [0]

    def nm(pfx):
        uid[0] += 1
        return f"{pfx}{uid[0]}"

    with TileContext(nc) as tc:
        with (
            tc.tile_pool(name="const", bufs=1) as constp,
            tc.tile_pool(name="wpool", bufs=1) as wpool,
            tc.tile_pool(name="xa", bufs=1) as xap,
            tc.tile_pool(name="psit", bufs=1, space="PSUM") as psi,
        ):
            ident = constp.tile([P, P], F16)
            make_identity(nc, ident)
            # shift-by-one-partition matrices: S_up moves partition p-1's
            # data to p (top halo), S_dn the reverse
            S_up = constp.tile([P, P], F16, tag="sup")
            S_dn = constp.tile([P, P], F16, tag="sdn")
            for tile_, base in ((S_up, 1), (S_dn, -1)):
                nc.gpsimd.memset(tile_, 0.0)
                nc.gpsimd.affine_select(
                    out=tile_, in_=tile_,
                    compare_op=mybir.AluOpType.not_equal,
                    fill=1.0, base=base, pattern=[[-1, P]],
                    channel_multiplier=1,
                )

            W_ALL = wpool.tile([P, 25, R, W], F16, tag="wall", name="wall")
            XA = xap.tile([P, ROWB, COLB], F16, tag="XA")

            def slot3(s):
                return W_ALL[:, s]

            def slotf(s):
                return W_ALL[:, s].rearrange("p a b -> p (a b)")

            def group_win(Xc, dh, dw0, step, ntap):
                """[P, ntap, R, W] AP over the x buffer: tap t reads the
                window shifted by (dh, dw0 + t*step)."""
                w = Xc[:, HALO + dh:HALO + dh + R, CB + dw0:CB + dw0 + W]
                return bass.AP(
                    w.tensor, w.offset,
                    [list(w.ap[0]), [step, ntap], [COLB, R], [1, W]],
                )

            mm_n = [0]

            def acc(ps, mflat, nchunk):
                """Accumulate nchunk 512-col chunks of mflat into ps.
                The first 5 chunks of an iteration (one per PSUM bank
                column) reset the accumulation (start=True); the stop
                is always emitted by emit_tail_fused."""
                for k in range(nchunk):
                    first = mm_n[0] < 5
                    pcol = (k * 512) % RW
                    nc.tensor.matmul(
                        out=ps[:, pcol:pcol + 512],
                        lhsT=ident,
                        rhs=mflat[:, k * 512:(k + 1) * 512],
                        start=first,
                        stop=False,
                    )
                    mm_n[0] += 1

            HB3 = HALO * COLB      # 1944, halo span per side
            HB = RW // 2           # 1280, half-field

            def emit_tail_fused(mflat, ntap, ps, Xn, last_iter, shiftp):
                """Emit the last (single-tap center) group's 5 matmuls,
                evacuate PSUM on Scalar (PSUM readers on different
                engines serialize anyway, so one full copy beats two
                halves), then rebuild the halo rows via PE partition
                shifts in 512-col units double-buffered through two
                1-bank psh tiles.  PSUM WAR tracking is tile-granular,
                so the evacuation strictly follows all matmuls -- keep
                the matmul count after the last tap product minimal."""
                for b in range(5):
                    for t in range(ntap):
                        nc.tensor.matmul(
                            out=ps[:, b * 512:(b + 1) * 512],
                            lhsT=ident,
                            rhs=mflat[:, t * RW + b * 512:
                                      t * RW + (b + 1) * 512],
                            start=False,
                            stop=(b == 4 and t == ntap - 1),
                        )
                        mm_n[0] += 1
                if last_iter:
                    return
                nc.scalar.copy(
                    out=Xn[:, HALO:HALO + R, CB:CB + W],
                    in_=ps.rearrange("p (a b) -> p a b", a=R),
                )
                Xn_f = Xn.rearrange("p a b -> p (a b)")
                for S, src0, dst0 in (
                    (S_up, (HALO + 1) * COLB, 0),
                    (S_dn, HALO * COLB, (R + HALO) * COLB),
                ):
                    for c0 in range(0, HB3, 512):
                        ln = min(512, HB3 - c0)
                        psh = shiftp.tile([P, 512], F32, tag="sh",
                                          name=nm("sh_"))
                        nc.tensor.matmul(
                            out=psh[:, 0:ln], lhsT=S,
                            rhs=Xn_f[:, src0 + c0:src0 + c0 + ln],
                            start=True, stop=True,
                        )
                        nc.scalar.copy(
                            out=Xn_f[:, dst0 + c0:dst0 + c0 + ln],
                            in_=psh[:, 0:ln],
                        )

            # ---------------- setup + iteration 1 ----------------
            ps0 = psi.tile([P, RW], F32, tag="ps", name="ps_it0")
            with (
                tc.tile_pool(name="syncg", bufs=1) as sgp,
                tc.tile_pool(name="gpg", bufs=4) as ggp,
                tc.tile_pool(name="fusep", bufs=1) as fusep,
                tc.tile_pool(name="m0pool", bufs=3) as m0pool,
                tc.tile_pool(name="psst", bufs=1, space="PSUM") as psp,
            ):
                nc.vector.memset(XA, 0.0)
                for d_idx in range(3):
                    d = d_idx + 1
                    dil_slots = [FIELD_OF[((ch // 3 - 1) * d, (ch % 3 - 1) * d)]
                                 for ch in range(9) if ch != 4]

                    fuse16 = fusep.tile([P, RW], F16, tag="f16",
                                        name=nm("f16_"))
                    nc.gpsimd.dma_start(
                        out=fuse16, in_=rows_ap(fuse_dram, d_idx * H * W)
                    )
                    e_c = fusep.tile([P, R, W], F16, tag="ec", name=nm("ec_"))
                    for ch in range(9):
                        if ch == 4:
                            dest = e_c.rearrange("p a b -> p (a b)")
                        else:
                            s = FIELD_OF[((ch // 3 - 1) * d, (ch % 3 - 1) * d)]
                            dest = slotf(s)
                        g = ggp.tile([P, RW], F16, tag="g16",
                                     name=nm("g16_"))
                        nc.gpsimd.dma_start(
                            out=g, in_=rows_ap(g_dram[d_idx], ch * H * W)
                        )
                        nc.scalar.activation(
                            out=dest, in_=g,
                            func=mybir.ActivationFunctionType.Exp,
                        )
                    if d_idx == 0:
                        # x load + halo-init ride the idle Act HWDGE ring
                        xs32 = sgp.tile([P, RW], F32, tag="g")
                        nc.scalar.dma_start(out=xs32, in_=rows_ap(x_dram))
                        nc.vector.tensor_copy(
                            out=XA[:, HALO:HALO + R, CB:CB + W],
                            in_=xs32.rearrange("p (a b) -> p a b", a=R),
                        )
                        XA_f = XA.rearrange("p a b -> p (a b)")
                        nc.scalar.dma_start(
                            out=XA_f[1:P, 0:HALO * COLB],
                            in_=XA_f[0:P - 1, R * COLB:(R + HALO) * COLB],
                        )
                        nc.scalar.dma_start(
                            out=XA_f[0:P - 1, (R + HALO) * COLB:NFLAT],
                            in_=XA_f[1:P, HALO * COLB:2 * HALO * COLB],
                        )
                    # channel sums + 1/sum in halves (3 PSUM banks)
                    t_ = fusep.tile([P, RW], F16, tag="t", name=nm("t_"))
                    for h0 in (0, HB):
                        pss = psp.tile([P, HB], F32, tag="pss",
                                       name=nm("pss_"))
                        for ci, ch in enumerate(range(9)):
                            if ch == 4:
                                sf = e_c.rearrange("p a b -> p (a b)")
                            else:
                                sf = slotf(
                                    FIELD_OF[((ch // 3 - 1) * d,
                                              (ch % 3 - 1) * d)])
                            for c0 in range(0, HB, 512):
                                c1 = min(c0 + 512, HB)
                                nc.tensor.matmul(
                                    out=pss[:, c0:c1], lhsT=ident,
                                    rhs=sf[:, h0 + c0:h0 + c1],
                                    start=(ci == 0), stop=(ci == 8),
                                )
                        r_ = fusep.tile([P, HB], F32, tag="r", name=nm("r_"))
                        nc.vector.reciprocal_approx_fast(out=r_, in_=pss)
                        nc.vector.tensor_mul(
                            out=t_[:, h0:h0 + HB], in0=fuse16[:, h0:h0 + HB],
                            in1=r_,
                        )
                    tv = t_.rearrange("p (a b) -> p a b", a=R)
                    for s in dil_slots:
                        nc.vector.tensor_mul(out=slot3(s), in0=slot3(s),
                                             in1=tv)
                    if d_idx == 0:
                        nc.vector.tensor_mul(out=slot3(CENTER_SLOT),
                                             in0=e_c, in1=tv)
                    else:
                        nc.vector.tensor_mul(out=e_c, in0=e_c, in1=tv)
                        nc.vector.tensor_add(out=slot3(CENTER_SLOT),
                                             in0=slot3(CENTER_SLOT), in1=e_c)
                    # iteration-1 taps of this dilation, single-tap ops:
                    # DMA-hidden, small m0 footprint.  The center tap is
                    # deferred to the iteration section's fused tail.
                    taps = [((ch // 3 - 1) * d, (ch % 3 - 1) * d)
                            for ch in range(9) if ch != 4]
                    for dh, dw in taps:
                        m0 = m0pool.tile([P, R, W], F16, tag="m",
                                         name=nm(f"m0d{d}_"))
                        win = XA[:, HALO + dh:HALO + dh + R,
                                 CB + dw:CB + dw + W]
                        nc.vector.tensor_mul(
                            out=m0, in0=slot3(FIELD_OF[(dh, dw)]), in1=win,
                        )
                        acc(ps0, m0.rearrange("p a b -> p (a b)"), 5)
                assert mm_n[0] == 120

            # ---------------- iterations 2..8 ----------------
            with (
                tc.tile_pool(name="xb", bufs=1) as xbp,
                tc.tile_pool(name="shp", bufs=2, space="PSUM") as shiftp,
            ):
                XB = xbp.tile([P, ROWB, COLB], F16, tag="XB")
                # only the border cols need zeroing (the interior is
                # overwritten by evacuation, halo rows by the halo DMAs)
                # -- plus the image-edge halo rows the DMAs never touch
                nc.vector.memset(XB[:, :, 0:CB], 0.0)
                nc.vector.memset(XB[:, :, CB + W:COLB], 0.0)

                with tc.tile_pool(name="mpool", bufs=3) as mpool:
                    # iteration-1's deferred center tap + fused tail
                    mc = mpool.tile([P, 3, R, W], F16, tag="m",
                                    name=nm("mc_"))
                    nc.vector.tensor_mul(
                        out=mc[:, 0],
                        in0=slot3(CENTER_SLOT),
                        in1=XA[:, HALO:HALO + R, CB:CB + W],
                    )
                    emit_tail_fused(
                        mc[:, 0:1].rearrange("p t a b -> p (t a b)"),
                        1, ps0, XB, False, shiftp)
                    assert mm_n[0] == 125

                    bufs = [XA, XB]
                    ps = ps0
                    for it in range(1, PROP_TIME):
                        Xc = bufs[it % 2]
                        Xn = bufs[(it + 1) % 2]
                        mm_n[0] = 0
                        ps = psi.tile([P, RW], F32, tag="ps",
                                      name=nm("ps_"))
                        for g in GROUP_ORDER:
                            dh, dw0, step, ntap, slot0 = GROUPS[g]
                            m = mpool.tile([P, 3, R, W], F16, tag="m",
                                           name=nm(f"m{it}_"))
                            msub = m[:, 0:ntap]
                            nc.vector.tensor_mul(
                                out=msub,
                                in0=W_ALL[:, slot0:slot0 + ntap],
                                in1=group_win(Xc, dh, dw0, step, ntap),
                            )
                            acc(ps,
                                msub.rearrange("p t a b -> p (t a b)"),
                                5 * ntap)
                        # last group: single-tap center -- only 5 matmuls
                        # separate its product from the evacuation
                        ml = mpool.tile([P, 3, R, W], F16, tag="m",
                                        name=nm(f"ml{it}_"))
                        nc.vector.tensor_mul(
                            out=ml[:, 0],
                            in0=slot3(CENTER_SLOT),
                            in1=Xc[:, HALO:HALO + R, CB:CB + W],
                        )
                        emit_tail_fused(
                            ml[:, 0:1].rearrange("p t a b -> p (t a b)"),
                            1, ps, Xn, it == PROP_TIME - 1, shiftp)
                        assert mm_n[0] == 125

                with tc.tile_pool(name="stagep", bufs=1) as stagep:
                    stage = stagep.tile([P, RW], F32)
                    nc.scalar.copy(out=stage, in_=ps)
                    nc.sync.dma_start(out=rows_ap(out_dram), in_=stage)

    nc.compile()
    return nc


_NC = None


def _get_nc():
    global _NC
    if _NC is None:
        _NC = build_nc()
    return _NC


def _in_maps(guided1, guided2, guided3, fuse, x):
    maps = []
    for b in range(NCORES):
        maps.append({
            "guided1": np.ascontiguousarray(guided1[b], dtype=np.float32),
            "guided2": np.ascontiguousarray(guided2[b], dtype=np.float32),
            "guided3": np.ascontiguousarray(guided3[b], dtype=np.float32),
            "fuse": np.ascontiguousarray(fuse[b], dtype=np.float32),
            "x": np.ascontiguousarray(x[b], dtype=np.float32),
        })
    return maps


def kernel(guided1, guided2, guided3, fuse, x):
    nc = _get_nc()
    res = run_bass_kernel_spmd(
        nc, _in_maps(guided1, guided2, guided3, fuse, x),
        core_ids=list(range(NCORES)),
    )
    return np.stack([res.results[b]["out"] for b in range(NCORES)], axis=0)


def kernel_profiled(guided1, guided2, guided3, fuse, x):
    """Returns (output, BassKernelResults) with trace enabled."""
    nc = _get_nc()
    res = run_bass_kernel_spmd(
        nc, _in_maps(guided1, guided2, guided3, fuse, x),
        core_ids=list(range(NCORES)), trace=True,
    )
    out = np.stack([res.results[b]["out"] for b in range(NCORES)], axis=0)
    return out, res
